# revision 1
# baseline (speedup 1.0000x reference)
"""Trainium2 Bass kernel for nn_Discriminator (2-layer LSTM, B=512 T=100 H=1024).

Strategy: data-parallel over batch across 8 cores (B=64 per core).
Per core:
  - Layer-1 input projections collapse through the encoder:
    A1 = x @ (W_ih1 @ enc_W)^T, a [6400,34]@[34,4096] matmul (K=34)
    instead of enc=[6400,1024] then [6400,1024]@[1024,4096].
    W_combT (+ gate bias row) is built incrementally from W_ih1 row
    chunks, so no 16MB transposed copy of W_ih1 is ever materialized.
  - Recurrence weights / h state / A scratch are bf16: half the DMA
    and SBUF traffic and lower PE power (the chip power-throttles the
    PE duty cycle under sustained all-engine load, so energy saved
    anywhere buys PE clock).
  - Gate bias + A-scratch are pre-added into PSUM before the gate
    matmuls (start=False), so activations read PSUM immediately after
    the last matmul.
"""

import numpy as np

import concourse.bass as bass
import concourse.tile as tile
import concourse.mybir as mybir
from concourse import bacc
from concourse.bass_utils import run_bass_kernel_spmd
from concourse.masks import make_identity

F32 = mybir.dt.float32
F32R = mybir.dt.float32r
BF16 = mybir.dt.bfloat16
AF = mybir.ActivationFunctionType

N_CORES = 8
B, IN, H = 512, 34, 1024
G = 4 * H  # 4096
BPC = B // N_CORES  # 64 batch rows per core
P = 128
KC = H // P  # 8 contraction chunks

WDT = BF16   # A2 weight dtype
ADT = BF16   # A1/A2 scratch dtype
HDT = BF16   # h^T ring dtype (flush / A2 path)
FP8 = mybir.dt.float8e4  # recurrence matmul dtype (DoubleRow)
KPF = KC // 2  # fp8 k-pairs per contraction


def _build_weight_T(nc, w_dram, w_T, identr, wrow, wtr_ps):
    """Transpose w_dram [G, H] into resident SBUF tile w_T [128, KC, G]
    (w_T[p, k, j] = W[j, 128k + p]) via PE transposes; w_T may be bf16
    (cast happens in the PSUM->SBUF copy)."""
    n_row_tiles = w_dram.shape[0] // P  # 32
    for r in range(n_row_tiles):
        wt = wrow.tile([P, H], F32R, tag="wrow")
        nc.sync.dma_start(wt[:], w_dram[r * P:(r + 1) * P, :].bitcast(F32R))
        for c in range(KC):
            pt = wtr_ps.tile([P, P], F32R, tag="wtr")
            nc.tensor.transpose(pt[:], wt[:, c * P:(c + 1) * P], identr[:])
            # gpsimd cannot touch PSUM; alternate vector / scalar(Copy)
            if c % 2 == 0:
                nc.vector.tensor_copy(w_T[:, c, r * P:(r + 1) * P], pt[:])
            else:
                nc.scalar.activation(w_T[:, c, r * P:(r + 1) * P], pt[:], AF.Copy)


def _build_weight_T8(nc, w_dram, w_T8, identr, wrow, wtr_ps):
    """Like _build_weight_T but emits fp8 [128, KPF, 2, G] (k-pair packed
    for DoubleRow): w_T8[p, kp, e, j] = W[j, 128*(2*kp+e) + p]."""
    n_row_tiles = w_dram.shape[0] // P  # 32
    for r in range(n_row_tiles):
        wt = wrow.tile([P, H], F32R, tag="wrow")
        nc.sync.dma_start(wt[:], w_dram[r * P:(r + 1) * P, :].bitcast(F32R))
        for c in range(KC):
            pt = wtr_ps.tile([P, P], F32R, tag="wtr")
            nc.tensor.transpose(pt[:], wt[:, c * P:(c + 1) * P], identr[:])
            dst = w_T8[:, c // 2, c % 2, r * P:(r + 1) * P]
            if c % 2 == 0:
                nc.vector.tensor_copy(dst, pt[:])
            else:
                nc.scalar.activation(dst, pt[:], AF.Copy)


def _emit_A2_phase(nc, T, *, w_T, bias128, lhs_blk_fn, a_dram_flat, psum_a, ev_pool):
    """A2 = lhs @ W^T (no bias), lhs supplied per 256-column block by
    lhs_blk_fn (returns SBUF tile [128, KC, 256] = lhs^T block).
    Output rows are (t*BPC + b) flattened, written [T*BPC, G] bf16."""
    n_blocks = (T * BPC) // 256
    for mb in range(n_blocks):
        lhsT_blk = lhs_blk_fn(mb)
        for mt in range(2):
            row0 = mb * 256 + mt * P
            for np_ in range(4):  # pairs of 512-wide n chunks
                pts = [psum_a.tile([P, 512], F32, tag="pa", name=f"pa{j}")
                       for j in range(2)]
                for k in range(KC):
                    for j in range(2):
                        n = np_ * 2 + j
                        nc.tensor.matmul(
                            pts[j][:],
                            lhsT_blk[:, k, mt * P:(mt + 1) * P],
                            w_T[:, k, n * 512:(n + 1) * 512],
                            start=(k == 0), stop=(k == KC - 1),
                        )
                for j in range(2):
                    n = np_ * 2 + j
                    ev = ev_pool.tile([P, 512], ADT, tag="aev")
                    nc.vector.tensor_add(ev[:], pts[j][:],
                                         bias128[:, n * 512:(n + 1) * 512])
                    nc.sync.dma_start(
                        a_dram_flat[row0:row0 + P, n * 512:(n + 1) * 512], ev[:]
                    )


def _emit_recurrence(nc, T, *, a_dram, w_T8, hT8, hTb, c_t, a_pool,
                     gact_pool, hpool, psum_g, psum_tr, misc_pool,
                     h1T_dram=None, dec=None, out_ap=None, ident=None,
                     identh=None, xw=None):
    """T sequential LSTM-cell steps for one layer.

    hT8: fp8 ring [128, KPF, 2, 8, BPC] (k-pair packed, DoubleRow lhsT);
    step t writes slot t%8, reads slot (t-1)%8.  hTb: optional bf16 ring
    [128, KC, 8, BPC] kept in parallel for the h1T flush / A2 path.
    c_t: [BPC, H] fp32 persistent cell state.
    Gate preactivation = PSUM preload (a_t + bias) + h @ W_hh^T (fp8
    DoubleRow: 4 matmuls per 512-wide gate half).
    Weight gate order along G: i, f, g, o.
    """
    a_tiles = {}

    def load_a(t):
        if a_dram is None:
            return
        a1t = a_pool.tile([BPC, G], ADT, tag="a1t", name=f"a1t_{t % 4}")
        nc.sync.dma_start(a1t[:], a_dram[t])
        a_tiles[t] = a1t

    load_a(0)
    pg_next = {}

    for t in range(T):
        s_r = (t + 7) % 8
        s_w = t % 8
        if t + 1 < T:
            load_a(t + 1)
        a1t = a_tiles.get(t)

        acts = {}

        def preload_pe(g_idx, pg, at, tt=None):
            # layer 1 (xw): a_t computed on the fly as x_t @ W_comb^T with
            # the bias riding as contraction row IN (ones row in xT) — no
            # A-scratch, no DMA. Layer 2: identity matmul injects a2_t.
            for n2 in range(2):
                n = g_idx * 2 + n2
                if xw is not None:
                    xT, wcT = xw
                    nc.tensor.matmul(
                        pg[:, n2 * 512:(n2 + 1) * 512],
                        xT[:, tt * BPC:(tt + 1) * BPC],
                        wcT[:, n * 512:(n + 1) * 512],
                        start=True, stop=False, skip_group_check=True,
                    )
                else:
                    nc.tensor.matmul(
                        pg[:, n2 * 512:(n2 + 1) * 512],
                        identh[:BPC, :BPC],
                        at[:, n * 512:(n + 1) * 512],
                        start=True, stop=False, skip_group_check=True,
                    )

        def mm_gate(g_idx, pg, n2_outer=False, preloaded=False):
            if not preloaded:
                preload_pe(g_idx, pg, a1t, tt=t)
            loops = ([(n2, kp) for n2 in (1, 0) for kp in range(KPF)] if n2_outer
                     else [(n2, kp) for kp in range(KPF) for n2 in range(2)])
            for n2, kp in loops:
                n = g_idx * 2 + n2
                nc.tensor.matmul(
                    pg[:, n2 * 512:(n2 + 1) * 512],
                    hT8[:, kp, :, s_r, :],
                    w_T8[:, kp, :, n * 512:(n + 1) * 512],
                    start=False, stop=(kp == KPF - 1),
                    perf_mode=mybir.MatmulPerfMode.DoubleRow,
                    skip_group_check=True,
                )

        def do_gate(g_idx, func, tag):
            pg = pg_next.pop(g_idx, None)
            if pg is None:
                pg = psum_g.tile([BPC, H], F32, tag="pg", name=f"pg{g_idx}")
                mm_gate(g_idx, pg)
            else:
                mm_gate(g_idx, pg, preloaded=True)
            at = gact_pool.tile([BPC, H], HDT, tag="gact", name=tag)
            nc.scalar.activation(at[:], pg[:], func)
            acts[g_idx] = at

        HF = 512  # half of H, processed separately to shorten the serial tail
        do_gate(0, AF.Sigmoid, "act_i")        # input gate
        do_gate(2, AF.Tanh, "act_g")           # candidate
        tmp = gact_pool.tile([BPC, H], HDT, tag="gact", name="tmp")
        nc.vector.tensor_mul(tmp[:], acts[0][:], acts[2][:])

        pg_f = psum_g.tile([BPC, H], F32, tag="pg", name="pg_f")
        mm_gate(1, pg_f, n2_outer=True)
        act_f = gact_pool.tile([BPC, H], HDT, tag="gact", name="act_f")
        tanh_c = gact_pool.tile([BPC, H], HDT, tag="gact", name="tanh_c")
        for hh in (1, 0):
            sl = slice(hh * HF, (hh + 1) * HF)
            nc.scalar.activation(act_f[:, sl], pg_f[:, sl], AF.Sigmoid)
            nc.vector.tensor_mul(c_t[:, sl], c_t[:, sl], act_f[:, sl])
            nc.vector.tensor_add(c_t[:, sl], c_t[:, sl], tmp[:, sl])
            nc.scalar.activation(tanh_c[:, sl], c_t[:, sl], AF.Tanh)

        # keepalive: the PE drops to the 1.2GHz p-state during the tail idle
        # and takes ~10 matmuls to recover; a tiny matmul keeps it hot
        ka1 = psum_tr.tile([1, 256], F32, tag="htr", name="ka1")
        nc.tensor.matmul(ka1[:], identh[:BPC, 0:1], tanh_c[:, 0:256],
                         start=True, stop=True)

        # output gate + h + h^T, in halves so hT chunks stream out early
        pg_o = psum_g.tile([BPC, H], F32, tag="pg", name="pg_o")
        mm_gate(3, pg_o, n2_outer=True)
        act_o = gact_pool.tile([BPC, H], HDT, tag="gact", name="act_o")
        h_t = hpool.tile([BPC, H], HDT, tag="h_t")
        for hh in (1, 0):
            sl = slice(hh * HF, (hh + 1) * HF)
            nc.scalar.activation(act_o[:, sl], pg_o[:, sl], AF.Sigmoid)
            nc.vector.tensor_mul(h_t[:, sl], act_o[:, sl], tanh_c[:, sl])
        ka2 = psum_tr.tile([1, 256], F32, tag="htr", name="ka2")
        nc.tensor.matmul(ka2[:], identh[:BPC, 0:1], tanh_c[:, 256:512],
                         start=True, stop=True)

        # gate-i preload of step t+1 rides before the transposes: the PE
        # does it while waiting on h_t, and the post-transpose restart goes
        # straight into DoubleRow matmuls
        if t + 1 < T:
            pg_i = psum_g.tile([BPC, H], F32, tag="pg", name="pg0")
            preload_pe(0, pg_i, a_tiles.get(t + 1), tt=t + 1)
            pg_next[0] = pg_i
            pg_g = psum_g.tile([BPC, H], F32, tag="pg", name="pg2")
            preload_pe(2, pg_g, a_tiles.get(t + 1), tt=t + 1)
            pg_next[2] = pg_g

        # reversed: chunk 0 (needed first by next step) lands last, so the
        # scheduler cannot interleave next-step matmuls with the transposes
        for k in range(KC - 1, -1, -1):
            pt = psum_tr.tile([P, BPC], HDT, tag="htr")
            nc.tensor.transpose(pt[:], h_t[:, k * P:(k + 1) * P], identh[:BPC, :BPC])
            if hTb is not None:
                # critical fp8 ring via vector; bf16 flush/A2 ring via the
                # scalar engine (both read the transpose PSUM in parallel)
                nc.vector.tensor_copy(hT8[:, k // 2, k % 2, s_w, :], pt[:])
                nc.scalar.activation(hTb[:, k, s_w, :], pt[:], AF.Copy)
            else:
                nc.vector.tensor_copy(hT8[:, k // 2, k % 2, s_w, :], pt[:])
                if dec is not None and t == T - 1:
                    # bf16 copy of the final h2^T for a full-precision decode
                    nc.scalar.activation(dec[3][:, k, :], pt[:], AF.Copy)

        if h1T_dram is not None and s_w % 4 == 3:
            # half-ring flush every 4 steps: the ring slot being flushed is
            # rewritten 4 steps later, so the DMA read never stalls the next
            # step's ring copies (an 8-step flush rewrites slot 0 immediately)
            s0 = s_w - 3
            nc.sync.dma_start(
                h1T_dram.rearrange("(c p) n -> p c n", p=P)[:, :, (t - 3) * BPC:(t + 1) * BPC],
                hTb[:, :, s0:s0 + 4, :],
            )

        if dec is not None and t == T - 1:
            decWT, decb_sb, ones_bpc, hT_last = dec
            pd = psum_g.tile([1, BPC], F32, tag="pg", name="pdec")
            for k in range(KC):
                nc.tensor.matmul(pd[:], decWT[:, k:k + 1],
                                 hT_last[:, k, :],
                                 start=(k == 0), stop=False)
            nc.tensor.matmul(pd[:], decb_sb[:], ones_bpc[:],
                             start=False, stop=True)
            osb = misc_pool.tile([1, BPC], F32, tag="osb")
            nc.vector.tensor_copy(osb[:], pd[:])
            nc.sync.dma_start(out_ap.rearrange("b o -> o b"), osb[:])


def build(T=100):
    nc = bacc.Bacc("TRN2", target_bir_lowering=False, debug=False,
                   num_devices=N_CORES)

    x = nc.dram_tensor("x", [BPC, T, IN], F32, kind="ExternalInput").ap()
    enc_W = nc.dram_tensor("enc_W", [H, IN], F32, kind="ExternalInput").ap()
    enc_b = nc.dram_tensor("enc_b", [H], F32, kind="ExternalInput").ap()
    W_ih1 = nc.dram_tensor("W_ih1", [G, H], F32, kind="ExternalInput").ap()
    W_hh1 = nc.dram_tensor("W_hh1", [G, H], F32, kind="ExternalInput").ap()
    b_ih1 = nc.dram_tensor("b_ih1", [G], F32, kind="ExternalInput").ap()
    b_hh1 = nc.dram_tensor("b_hh1", [G], F32, kind="ExternalInput").ap()
    W_ih2 = nc.dram_tensor("W_ih2", [G, H], F32, kind="ExternalInput").ap()
    W_hh2 = nc.dram_tensor("W_hh2", [G, H], F32, kind="ExternalInput").ap()
    b_ih2 = nc.dram_tensor("b_ih2", [G], F32, kind="ExternalInput").ap()
    b_hh2 = nc.dram_tensor("b_hh2", [G], F32, kind="ExternalInput").ap()
    dec_W = nc.dram_tensor("dec_W", [1, H], F32, kind="ExternalInput").ap()
    dec_b = nc.dram_tensor("dec_b", [1], F32, kind="ExternalInput").ap()
    out = nc.dram_tensor("out", [BPC, 1], F32, kind="ExternalOutput").ap()

    A1 = nc.dram_tensor("A1_scratch", [T, BPC, G], ADT).ap()
    A2 = nc.dram_tensor("A2_scratch", [T, BPC, G], ADT).ap()
    h1T = nc.dram_tensor("h1T_scratch", [H, T * BPC], HDT).ap()
    A1_flat = A1.rearrange("t b g -> (t b) g")
    A2_flat = A2.rearrange("t b g -> (t b) g")

    with tile.TileContext(nc) as tc:
        with tc.tile_pool(name="persist", bufs=1) as persist, \
             tc.tile_pool(name="state", bufs=1) as state, \
             tc.tile_pool(name="misc", bufs=1) as misc:

            ident = persist.tile([P, P], F32, tag="ident")
            make_identity(nc, ident[:])
            identr = persist.tile([P, P], F32R, tag="identr")
            nc.vector.tensor_copy(identr[:], ident[:])
            identh = persist.tile([P, P], HDT, tag="identh")
            nc.vector.tensor_copy(identh[:], ident[:])
            ones1 = persist.tile([1, P], F32R, tag="ones1")
            nc.gpsimd.memset(ones1[:].bitcast(F32), 1.0)

            # persistent layer-1 input-side operands: row IN of xT is all
            # ones and row IN of W_combT is the gate-bias row, so the K=35
            # preload matmul x_t @ W_comb^T lands A1+bias in one pass
            W_combT = persist.tile([IN + 1, G], F32R, tag="W_combT")
            xT = persist.tile([IN + 1, T * BPC], F32R, tag="xT")
            bias128_2 = persist.tile([P, G], ADT, tag="bias128_2")

            def bcast_bias_row(brow, dst, ps_pool):
                """dst[p, n*512:(n+1)*512] = brow[0, n*512:...] for all p."""
                for n in range(8):
                    sl = slice(n * 512, (n + 1) * 512)
                    pb2 = ps_pool.tile([P, 512], F32, tag="pbb")
                    nc.tensor.matmul(pb2[:], ones1[:], brow[:, sl],
                                     start=True, stop=True)
                    nc.vector.tensor_copy(dst[:, sl], pb2[:])

            # ============ Phase E: xT [IN+1, T*BPC] (row IN = ones) ======
            if True:
                with nc.named_scope("phaseE"):
                    with tc.tile_pool(name="e_sb", bufs=3) as e_sb, \
                         tc.tile_pool(name="e_ps", bufs=3, space="PSUM") as e_ps:
                        onesrow = e_sb.tile([1, T * BPC], F32R, tag="onesrow")
                        nc.gpsimd.memset(onesrow[:].bitcast(F32), 1.0)
                        # DMA (not an engine op) may target the unaligned
                        # partition offset IN=34
                        nc.sync.dma_start(xT[IN:IN + 1, :], onesrow[:])
                        xr = x.rearrange("b t f -> t b f")
                        for m in range((T * BPC) // P):
                            xt_ = e_sb.tile([P, IN], F32R, tag="xtile")
                            nc.sync.dma_start(xt_[:BPC, :], xr[2 * m].bitcast(F32R))
                            nc.sync.dma_start(xt_[BPC:, :], xr[2 * m + 1].bitcast(F32R))
                            pt = e_ps.tile([IN, P], F32R, tag="xtr")
                            nc.tensor.transpose(pt[:], xt_[:], identr[:])
                            nc.vector.tensor_copy(xT[0:IN, m * P:(m + 1) * P], pt[:])

                # ============ W_combT build (incremental, no 16MB W_ih1^T) ====
                # encwb [128, KC, IN+1]: cols 0..IN-1 = enc_W chunk rows,
                # col IN = enc_b chunk. A single lhsT gives both W_combT rows
                # and the enc_b @ W_ih1^T bias row in one PSUM pass.
                with nc.named_scope("build_Wcomb"):
                    with tc.tile_pool(name="wc_sb", bufs=1) as wc_sb, \
                         tc.tile_pool(name="wc_row", bufs=6) as wc_row, \
                         tc.tile_pool(name="wc_st", bufs=2) as wc_st, \
                         tc.tile_pool(name="wc_ps", bufs=2, space="PSUM") as wc_ps, \
                         tc.tile_pool(name="wc_ps2", bufs=1, space="PSUM") as wc_ps2:
                        encwb = wc_sb.tile([P, KC, IN], F32R, tag="encwb")
                        nc.sync.dma_start(
                            encwb[:],
                            enc_W.rearrange("(c p) f -> p c f", p=P).bitcast(F32R))
                        encb_k = wc_sb.tile([P, KC], F32R, tag="encb_k")
                        nc.sync.dma_start(
                            encb_k[:],
                            enc_b.rearrange("(c p) -> p c", p=P).bitcast(F32R))
                        brow1 = wc_sb.tile([1, G], F32R, tag="brow1")
                        bi1 = wc_sb.tile([1, G], F32, tag="bi1")
                        nc.sync.dma_start(bi1[:], b_ih1[None, :])
                        bh1 = wc_sb.tile([1, G], F32, tag="bh1")
                        nc.sync.dma_start(bh1[:], b_hh1[None, :])
                        # groups of 4 row-chunks = 512 G columns
                        for grp in range(G // 512):
                            wstage = wc_st.tile([P, KC, 512], F32R, tag="wstage")
                            for rr in range(4):
                                r = grp * 4 + rr
                                wt = wc_row.tile([P, H], F32R, tag="wcrow")
                                nc.sync.dma_start(
                                    wt[:], W_ih1[r * P:(r + 1) * P, :].bitcast(F32R))
                                for c in range(KC):
                                    ptr = wc_ps.tile([P, P], F32R, tag="wctr")
                                    nc.tensor.transpose(
                                        ptr[:], wt[:, c * P:(c + 1) * P], identr[:])
                                    if c % 2 == 0:
                                        nc.vector.tensor_copy(
                                            wstage[:, c, rr * P:(rr + 1) * P], ptr[:])
                                    else:
                                        nc.scalar.activation(
                                            wstage[:, c, rr * P:(rr + 1) * P],
                                            ptr[:], AF.Copy)
                            pb = wc_ps2.tile([IN, 512], F32, tag="wcpb")
                            pbias = wc_ps2.tile([1, 512], F32, tag="wcpbias")
                            for k in range(KC):
                                nc.tensor.matmul(pb[:], encwb[:, k, :],
                                                 wstage[:, k, :],
                                                 start=(k == 0), stop=(k == KC - 1))
                            for k in range(KC):
                                nc.tensor.matmul(pbias[:], encb_k[:, k:k + 1],
                                                 wstage[:, k, :],
                                                 start=(k == 0), stop=(k == KC - 1))
                            sl = slice(grp * 512, (grp + 1) * 512)
                            nc.vector.tensor_copy(W_combT[0:IN, sl], pb[:])
                            # bias row = enc_b@W^T + b_ih1 + b_hh1
                            nc.vector.tensor_add(brow1[:, sl], pbias[:],
                                                 bi1[:, sl])
                            nc.gpsimd.tensor_add(brow1[:, sl], brow1[:, sl],
                                                 bh1[:, sl])
                        # bias row rides as contraction row IN (DMA can hit
                        # the unaligned partition offset)
                        nc.sync.dma_start(W_combT[IN:IN + 1, :], brow1[:])

            # ============ W_hh1^T build + Phase R1 ============
            with tc.tile_pool(name="wpool", bufs=1) as wpool:
                w_T1 = wpool.tile([P, KPF, 2, G], FP8, tag="W8")
                with nc.named_scope("build_Whh1T"):
                    with tc.tile_pool(name="wrow1", bufs=6) as wrow, \
                         tc.tile_pool(name="wtr_ps1", bufs=3, space="PSUM") as wtr_ps:
                        _build_weight_T8(nc, W_hh1, w_T1, identr, wrow, wtr_ps)

                hT8 = state.tile([P, KPF, 2, 8, BPC], FP8, tag="hT8_ring")
                hTb = state.tile([P, KC, 8, BPC], HDT, tag="hTb_ring")
                c_t = state.tile([BPC, H], F32, tag="c_t")
                nc.gpsimd.memset(hT8[:].bitcast(mybir.dt.uint8), 0.0)
                nc.gpsimd.memset(hTb[:].bitcast(mybir.dt.uint16), 0.0)
                nc.gpsimd.memset(c_t[:], 0.0)

                with nc.named_scope("phaseR1"):
                    with tc.tile_pool(name="r1_a", bufs=2) as a_pool, \
                         tc.tile_pool(name="r1_g", bufs=4) as gact_pool, \
                         tc.tile_pool(name="r1_h", bufs=1) as hpool, \
                         tc.tile_pool(name="r1_pg", bufs=3, space="PSUM") as psum_g, \
                         tc.tile_pool(name="r1_ptr", bufs=2, space="PSUM") as psum_tr:
                        _emit_recurrence(nc, T, a_dram=None, xw=(xT, W_combT),
                                         w_T8=w_T1, hT8=hT8, hTb=hTb, c_t=c_t,
                                         a_pool=a_pool, gact_pool=gact_pool,
                                         hpool=hpool, psum_g=psum_g,
                                         psum_tr=psum_tr, misc_pool=misc,
                                         h1T_dram=h1T, ident=ident,
                                         identh=identh)

                # ============ W_ih2^T build + Phase A2 ============
                w_T2 = wpool.tile([P, KC, G], WDT, tag="W")
                with nc.named_scope("build_Wih2T"):
                    with tc.tile_pool(name="wrow2", bufs=6) as wrow, \
                         tc.tile_pool(name="wtr_ps2", bufs=3, space="PSUM") as wtr_ps:
                        _build_weight_T(nc, W_ih2, w_T2, identr, wrow, wtr_ps)

                with nc.named_scope("phaseA2"):
                    # layer-2 bias row: b_ih2 + b_hh2 broadcast
                    with tc.tile_pool(name="b2_sb", bufs=2) as b2_sb, \
                         tc.tile_pool(name="b2_ps", bufs=2, space="PSUM") as b2_ps:
                        for n in range(8):
                            slb = slice(n * 512, (n + 1) * 512)
                            bi2 = b2_sb.tile([1, 512], F32, tag="bi2")
                            nc.sync.dma_start(bi2[:], b_ih2[None, slb])
                            bh2 = b2_sb.tile([1, 512], F32, tag="bh2")
                            nc.sync.dma_start(bh2[:], b_hh2[None, slb])
                            brow2 = b2_sb.tile([1, 512], F32R, tag="brow2")
                            nc.vector.tensor_add(brow2[:], bi2[:], bh2[:])
                            pb2 = b2_ps.tile([P, 512], F32, tag="pbb")
                            nc.tensor.matmul(pb2[:], ones1[:], brow2[:],
                                             start=True, stop=True)
                            nc.vector.tensor_copy(bias128_2[:, slb], pb2[:])

                    h1T_r = h1T.rearrange("(c p) n -> p c n", p=P)
                    with tc.tile_pool(name="h1_sb", bufs=3) as h1_sb, \
                         tc.tile_pool(name="a_ps2", bufs=4, space="PSUM") as a_ps, \
                         tc.tile_pool(name="a_ev2", bufs=2) as a_ev:

                        def h1_blk(mb):
                            blk = h1_sb.tile([P, KC, 256], HDT, tag="h1blk")
                            nc.sync.dma_start(
                                blk[:], h1T_r[:, :, mb * 256:(mb + 1) * 256])
                            return blk

                        _emit_A2_phase(nc, T, w_T=w_T2, bias128=bias128_2,
                                       lhs_blk_fn=h1_blk,
                                       a_dram_flat=A2_flat,
                                       psum_a=a_ps, ev_pool=a_ev)

                # ============ W_hh2^T build + Phase R2 (+decode) ============
                w_T3 = wpool.tile([P, KPF, 2, G], FP8, tag="W8")
                with nc.named_scope("build_Whh2T"):
                    with tc.tile_pool(name="wrow3", bufs=6) as wrow, \
                         tc.tile_pool(name="wtr_ps3", bufs=3, space="PSUM") as wtr_ps:
                        _build_weight_T8(nc, W_hh2, w_T3, identr, wrow, wtr_ps)

                nc.gpsimd.memset(hT8[:].bitcast(mybir.dt.uint8), 0.0)
                nc.gpsimd.memset(c_t[:], 0.0)

                decWT_f = misc.tile([P, KC], F32, tag="decWT_f")
                nc.sync.dma_start(decWT_f[:], dec_W.rearrange("o (c p) -> p (c o)", p=P))
                decWT = misc.tile([P, KC], HDT, tag="decWT")
                nc.vector.tensor_copy(decWT[:], decWT_f[:])
                decb_f = misc.tile([1, 1], F32, tag="decb_f")
                nc.sync.dma_start(decb_f[:], dec_b[None, :])
                decb_sb = misc.tile([1, 1], HDT, tag="decb")
                nc.vector.tensor_copy(decb_sb[:], decb_f[:])
                ones_f = misc.tile([1, BPC], F32, tag="ones_f")
                nc.gpsimd.memset(ones_f[:], 1.0)
                ones_bpc = misc.tile([1, BPC], HDT, tag="ones_bpc")
                hT_last = misc.tile([P, KC, BPC], HDT, tag="hT_last")
                nc.vector.tensor_copy(ones_bpc[:], ones_f[:])

                with nc.named_scope("phaseR2"):
                    with tc.tile_pool(name="r2_a", bufs=3) as a_pool, \
                         tc.tile_pool(name="r2_g", bufs=4) as gact_pool, \
                         tc.tile_pool(name="r2_h", bufs=1) as hpool, \
                         tc.tile_pool(name="r2_pg", bufs=3, space="PSUM") as psum_g, \
                         tc.tile_pool(name="r2_ptr", bufs=2, space="PSUM") as psum_tr:
                        _emit_recurrence(nc, T, a_dram=A2,
                                         w_T8=w_T3, hT8=hT8, hTb=None, c_t=c_t,
                                         a_pool=a_pool, gact_pool=gact_pool,
                                         hpool=hpool, psum_g=psum_g,
                                         psum_tr=psum_tr, misc_pool=misc,
                                         h1T_dram=None,
                                         dec=(decWT, decb_sb, ones_bpc, hT_last),
                                         out_ap=out, ident=ident, identh=identh)

    nc.compile()
    return nc


_cached_nc = None
_cached_fn = None  # (jitted shard_map fn, in_names, out_names, out_shapes, zeros)


def _build_jitted(nc):
    """Same lowering as bass2jax.run_bass_via_pjrt, but the jitted
    executable is cached so repeat kernel() calls skip recompilation."""
    import jax
    from jax.sharding import Mesh, PartitionSpec
    from jax.experimental.shard_map import shard_map
    from concourse import bass2jax, mybir as _mybir

    bass2jax.install_neuronx_cc_hook()
    partition_name = nc.partition_id_tensor.name if nc.partition_id_tensor else None
    in_names, out_names, out_avals, zero_outs = [], [], [], []
    for alloc in nc.m.functions[0].allocations:
        if not isinstance(alloc, _mybir.MemoryLocationSet):
            continue
        name = alloc.memorylocations[0].name
        if alloc.kind == "ExternalInput":
            if name != partition_name:
                in_names.append(name)
        elif alloc.kind == "ExternalOutput":
            shape = tuple(alloc.tensor_shape)
            dtype = _mybir.dt.np(alloc.dtype)
            out_names.append(name)
            out_avals.append(jax.core.ShapedArray(shape, dtype))
            zero_outs.append(np.zeros(shape, dtype))
    n_params = len(in_names)
    n_outs = len(out_avals)
    all_in_names = list(in_names) + list(out_names)
    if partition_name is not None:
        all_in_names.append(partition_name)
    donate = tuple(range(n_params, n_params + n_outs))

    def _body(*args):
        operands = list(args)
        if partition_name is not None:
            operands.append(bass2jax.partition_id_tensor())
        outs = bass2jax._bass_exec_p.bind(
            *operands,
            out_avals=tuple(out_avals),
            in_names=tuple(all_in_names),
            out_names=tuple(out_names),
            lowering_input_output_aliases=(),
            sim_require_finite=True,
            sim_require_nnan=True,
            nc=nc,
        )
        return tuple(outs)

    devices = jax.devices()[:N_CORES]
    mesh = Mesh(np.asarray(devices), ("core",))
    in_specs = (PartitionSpec("core"),) * (n_params + n_outs)
    out_specs = (PartitionSpec("core"),) * n_outs
    fn = jax.jit(
        shard_map(_body, mesh=mesh, in_specs=in_specs, out_specs=out_specs,
                  check_rep=False),
        donate_argnums=donate, keep_unused=True,
    )
    out_shapes = [a.shape for a in out_avals]
    return fn, in_names, out_names, out_shapes, zero_outs


_dev_cache = {}  # name -> (digest, device_array)


def _to_device(name, arr):
    """Replicate-concat a weight to all cores and keep it on device across
    calls (keyed by content hash) so repeat kernel() calls only ship x."""
    import hashlib
    import jax
    d = hashlib.blake2b(arr.tobytes(), digest_size=16).digest()
    hit = _dev_cache.get(name)
    if hit is not None and hit[0] == d:
        return hit[1]
    conc = np.concatenate([arr] * N_CORES, axis=0)
    darr = jax.device_put(conc)
    _dev_cache[name] = (d, darr)
    return darr


def kernel(**inputs):
    global _cached_nc, _cached_fn
    if _cached_nc is None:
        _cached_nc = build(100)
        _cached_fn = _build_jitted(_cached_nc)
    fn, in_names, out_names, out_shapes, zero_outs = _cached_fn
    ins = {k: np.ascontiguousarray(np.asarray(v, dtype=np.float32))
           for k, v in inputs.items()}
    concat_in = []
    for name in in_names:
        if name == "x":
            concat_in.append(ins["x"])  # already [512, T, IN]; axis0 shards
        else:
            concat_in.append(_to_device(name, ins[name]))
    i = out_names.index("out")
    last_err = None
    for attempt in range(3):
        try:
            concat_zeros = [np.zeros((N_CORES * z.shape[0], *z.shape[1:]), z.dtype)
                            for z in zero_outs]
            out_arrs = fn(*concat_in, *concat_zeros)
            outp = np.asarray(out_arrs[i]).reshape(B, 1)
            return outp.astype(np.float32)
        except Exception as e:  # transient NRT_EXEC_UNIT_UNRECOVERABLE etc.
            last_err = e
            _dev_cache.clear()
            concat_in = []
            for name in in_names:
                if name == "x":
                    concat_in.append(ins["x"])
                else:
                    concat_in.append(_to_device(name, ins[name]))
    raise last_err



# revision 17
# speedup vs baseline: 1.1409x; 1.1409x over previous
"""Trainium2 Bass kernel for nn_Discriminator (2-layer LSTM, B=512 T=100 H=1024).

Strategy: data-parallel over batch across 8 cores (B=64 per core), with
both LSTM layers FUSED into one software-pipelined loop:

  macro-step t computes layer-1 cell for time t and layer-2 cell for
  time t-1.  Gate preactivations accumulate in per-layer PSUM tiles
  (DoubleRow matmuls must write PSUM partition 0), but the activations
  write into ONE partition-stacked SBUF tile (L1 -> rows 0:64,
  L2 -> rows 64:128), so all downstream elementwise work (i*g, c
  update, tanh, h) and the h^T transposes run once on [128, .] tiles
  for both layers.  The PE always has ~13us of independent matmul work
  per step, so the serial activation chain hides completely and the PE
  clock stays at its 2.4GHz p-state instead of the ~1.2GHz it degrades
  to when the instruction stream has per-step gaps.

  - All three H-contraction products (h1@W_hh1^T, h1@W_ih2^T,
    h2@W_hh2^T) are fp8e4m3 DoubleRow matmuls reading one shared fp8
    h^T ring (columns 0:64 = h1, 64:128 = h2; the same slice feeds the
    W_hh1 and W_ih2 products).  Weights are scaled x32 before the fp8
    cast (U(+-0.031) weights are half-subnormal in e4m3 otherwise ->
    10-20% quantization error) and descaled for free via the
    activation-instruction `scale` operand.
  - Layer-1's input projection collapses through the encoder:
    W_comb = W_ih1 @ enc_W^T, and the per-step preload x_t @ W_comb^T
    rides a K=35 matmul whose lhsT carries [x_t ; ones]: the combined
    layer-1 bias lands with the projection.  Layer-2's bias is a K=1
    fp8 DoubleRow pair (ones x 32*b2), one 107ns matmul per 512-chunk.
  - No DRAM scratch at all; HBM traffic is weights + x (~67MB/core).
"""

import numpy as np

import concourse.bass as bass
import concourse.tile as tile
import concourse.mybir as mybir
from concourse import bacc
from concourse.bass_utils import run_bass_kernel_spmd
from concourse.masks import make_identity

F32 = mybir.dt.float32
F32R = mybir.dt.float32r
BF16 = mybir.dt.bfloat16
FP8 = mybir.dt.float8e4
AF = mybir.ActivationFunctionType
DR = mybir.MatmulPerfMode.DoubleRow

N_CORES = 8
B, IN, H = 512, 34, 1024
G = 4 * H                 # 4096
BPC = B // N_CORES        # 64 batch rows per core
P = 128
KC = H // P               # 8 contraction chunks
KPF = KC // 2             # 4 fp8 k-pairs
NSLOT = 4                 # h^T ring depth
HDT = BF16
HF = 512                  # half of H for the split serial tail

WSCALE = 32.0             # fp8 weight pre-scale (exact power of 2)
HSCALE = 16.0             # fp8 h-ring pre-scale (fixes subnormal h)
INV_WSCALE = 1.0 / (WSCALE * HSCALE)


def _build_weight_T8(nc, w_dram, w_T8, identr, wrow, wtr_ps):
    """Transpose w_dram [G, H] into resident fp8 SBUF tile
    w_T8 [128, KPF, 2, G] (k-pair packed for DoubleRow), scaled by
    WSCALE in the PSUM->SBUF eviction (the PE transpose datapath
    ignores the identity operand's values, so scaling must not ride
    the transpose itself)."""
    n_row_tiles = w_dram.shape[0] // P  # 32
    for r in range(n_row_tiles):
        wt = wrow.tile([P, H], F32R, tag="wrow")
        nc.sync.dma_start(wt[:], w_dram[r * P:(r + 1) * P, :].bitcast(F32R))
        for c in range(KC):
            pt = wtr_ps.tile([P, P], F32R, tag="wtr")
            nc.tensor.transpose(pt[:], wt[:, c * P:(c + 1) * P], identr[:])
            dst = w_T8[:, c // 2, c % 2, r * P:(r + 1) * P]
            # gpsimd cannot touch PSUM; alternate vector / scalar(Copy)
            if c % 2 == 0:
                nc.vector.tensor_scalar_mul(dst, pt[:], WSCALE)
            else:
                nc.scalar.activation(dst, pt[:], AF.Copy, scale=WSCALE)


def build(T=100):
    nc = bacc.Bacc("TRN2", target_bir_lowering=False, debug=False,
                   num_devices=N_CORES)

    x = nc.dram_tensor("x", [BPC, T, IN], F32, kind="ExternalInput").ap()
    enc_W = nc.dram_tensor("enc_W", [H, IN], F32, kind="ExternalInput").ap()
    enc_b = nc.dram_tensor("enc_b", [H], F32, kind="ExternalInput").ap()
    W_ih1 = nc.dram_tensor("W_ih1", [G, H], F32, kind="ExternalInput").ap()
    W_hh1 = nc.dram_tensor("W_hh1", [G, H], F32, kind="ExternalInput").ap()
    b_ih1 = nc.dram_tensor("b_ih1", [G], F32, kind="ExternalInput").ap()
    b_hh1 = nc.dram_tensor("b_hh1", [G], F32, kind="ExternalInput").ap()
    W_ih2 = nc.dram_tensor("W_ih2", [G, H], F32, kind="ExternalInput").ap()
    W_hh2 = nc.dram_tensor("W_hh2", [G, H], F32, kind="ExternalInput").ap()
    b_ih2 = nc.dram_tensor("b_ih2", [G], F32, kind="ExternalInput").ap()
    b_hh2 = nc.dram_tensor("b_hh2", [G], F32, kind="ExternalInput").ap()
    dec_W = nc.dram_tensor("dec_W", [1, H], F32, kind="ExternalInput").ap()
    dec_b = nc.dram_tensor("dec_b", [1], F32, kind="ExternalInput").ap()
    out = nc.dram_tensor("out", [BPC, 1], F32, kind="ExternalOutput").ap()

    with tile.TileContext(nc) as tc:
        with tc.tile_pool(name="persist", bufs=1) as persist, \
             tc.tile_pool(name="state", bufs=1) as state, \
             tc.tile_pool(name="misc", bufs=1) as misc:

            ident = persist.tile([P, P], F32, tag="ident")
            make_identity(nc, ident[:])
            identr = persist.tile([P, P], F32R, tag="identr")
            nc.vector.tensor_copy(identr[:], ident[:])
            identh = persist.tile([P, P], HDT, tag="identh")
            nc.vector.tensor_copy(identh[:], ident[:])
            zb = persist.tile([P, 1], F32, tag="zero_bias")
            nc.gpsimd.memset(zb[:], 0.0)

            # layer-1 input-side operands, pre-scaled by WSCALE so the
            # PSUM accumulation matches the fp8 DR product scale:
            #   W_combT [35, G]: rows 0:34 = 32*(W_ih1@enc_W)^T,
            #     row 34 = 32*(enc_b@W_ih1^T + b_ih1 + b_hh1)
            #   xTa [35, T, 64]: per step t: [x_t^T ; ones]
            W_combT = persist.tile([IN + 1, G], BF16, tag="W_combT")
            xTa = persist.tile([IN + 1, T, BPC], BF16, tag="xTa")
            ones1 = persist.tile([1, P], F32R, tag="ones1")
            nc.gpsimd.memset(ones1[:].bitcast(F32), 1.0)
            # layer-2 bias broadcast [128, G], pre-scaled by WSCALE*HSCALE
            bias128_2 = persist.tile([P, G], BF16, tag="bias128_2")

            # ============ Phase E: xTa ============
            with nc.named_scope("phaseE"):
                with tc.tile_pool(name="e_sb", bufs=3) as e_sb, \
                     tc.tile_pool(name="e_ps", bufs=3, space="PSUM") as e_ps:
                    onesrow = e_sb.tile([1, T, BPC], BF16, tag="onesrow")
                    nc.gpsimd.memset(onesrow[:], 1.0)
                    nc.sync.dma_start(xTa[IN:IN + 1, :, :], onesrow[:])
                    xr = x.rearrange("b t f -> t b f")
                    for m in range(T // 2):
                        xt_ = e_sb.tile([P, IN], F32R, tag="xtile")
                        nc.sync.dma_start(xt_[:BPC, :], xr[2 * m].bitcast(F32R))
                        nc.sync.dma_start(xt_[BPC:, :], xr[2 * m + 1].bitcast(F32R))
                        pt = e_ps.tile([IN, P], F32R, tag="xtr")
                        nc.tensor.transpose(pt[:], xt_[:], identr[:])
                        nc.vector.tensor_copy(xTa[0:IN, 2 * m, :], pt[:, 0:BPC])
                        nc.scalar.activation(xTa[0:IN, 2 * m + 1, :], pt[:, BPC:P], AF.Copy)

            # ============ W_combT build (incremental, scaled) ============
            with nc.named_scope("build_Wcomb"):
                with tc.tile_pool(name="wc_sb", bufs=1) as wc_sb, \
                     tc.tile_pool(name="wc_row", bufs=6) as wc_row, \
                     tc.tile_pool(name="wc_st", bufs=2) as wc_st, \
                     tc.tile_pool(name="wc_ps", bufs=2, space="PSUM") as wc_ps, \
                     tc.tile_pool(name="wc_ps2", bufs=1, space="PSUM") as wc_ps2:
                    encwb = wc_sb.tile([P, KC, IN], F32R, tag="encwb")
                    nc.sync.dma_start(
                        encwb[:],
                        enc_W.rearrange("(c p) f -> p c f", p=P).bitcast(F32R))
                    encb_k = wc_sb.tile([P, KC], F32R, tag="encb_k")
                    nc.sync.dma_start(
                        encb_k[:],
                        enc_b.rearrange("(c p) -> p c", p=P).bitcast(F32R))
                    # WSCALE rides the encoder-side operands of the
                    # W_comb contraction (transposes don't scale)
                    nc.vector.tensor_scalar_mul(encwb[:], encwb[:], WSCALE * HSCALE)
                    nc.vector.tensor_scalar_mul(encb_k[:], encb_k[:], WSCALE * HSCALE)
                    brow1 = wc_sb.tile([1, G], BF16, tag="brow1")
                    bsum1 = wc_sb.tile([1, G], F32, tag="bsum1")
                    # pre-scaled bias sums; transient pool so the [1, G]
                    # f32 scratch frees before the weight staging runs
                    with tc.tile_pool(name="wc_tmp", bufs=1) as wc_tmp:
                        tA = wc_tmp.tile([1, G], F32, tag="tA")
                        nc.sync.dma_start(tA[:], b_ih1[None, :])
                        tB = wc_tmp.tile([1, G], F32, tag="tB")
                        nc.sync.dma_start(tB[:], b_hh1[None, :])
                        nc.vector.tensor_add(bsum1[:], tA[:], tB[:])
                        nc.gpsimd.tensor_scalar_mul(bsum1[:], bsum1[:], WSCALE * HSCALE)
                        tA = wc_tmp.tile([1, G], F32, tag="tA")
                        nc.sync.dma_start(tA[:], b_ih2[None, :])
                        tB = wc_tmp.tile([1, G], F32, tag="tB")
                        nc.sync.dma_start(tB[:], b_hh2[None, :])
                        nc.vector.tensor_add(tA[:], tA[:], tB[:])
                        nc.gpsimd.tensor_scalar_mul(tA[:], tA[:],
                                                    WSCALE * HSCALE)
                        brow2 = wc_tmp.tile([1, G], F32R, tag="brow2")
                        nc.vector.tensor_copy(brow2[:], tA[:])
                        for n in range(8):
                            slb = slice(n * 512, (n + 1) * 512)
                            pbb = wc_ps.tile([P, 512], F32, tag="pbb")
                            nc.tensor.matmul(pbb[:], ones1[:], brow2[:, slb],
                                             start=True, stop=True)
                            nc.vector.tensor_copy(bias128_2[:, slb], pbb[:])
                    # groups of 4 row-chunks = 512 G columns
                    for grp in range(G // 512):
                        wstage = wc_st.tile([P, KC, 512], F32R, tag="wstage")
                        for rr in range(4):
                            r = grp * 4 + rr
                            wt = wc_row.tile([P, H], F32R, tag="wcrow")
                            nc.sync.dma_start(
                                wt[:], W_ih1[r * P:(r + 1) * P, :].bitcast(F32R))
                            for c in range(KC):
                                ptr = wc_ps.tile([P, P], F32R, tag="wctr")
                                nc.tensor.transpose(
                                    ptr[:], wt[:, c * P:(c + 1) * P], identr[:])
                                if c % 2 == 0:
                                    nc.vector.tensor_copy(
                                        wstage[:, c, rr * P:(rr + 1) * P], ptr[:])
                                else:
                                    nc.scalar.activation(
                                        wstage[:, c, rr * P:(rr + 1) * P],
                                        ptr[:], AF.Copy)
                        pb = wc_ps2.tile([IN, 512], F32, tag="wcpb")
                        pbias = wc_ps2.tile([1, 512], F32, tag="wcpbias")
                        for k in range(KC):
                            nc.tensor.matmul(pb[:], encwb[:, k, :],
                                             wstage[:, k, :],
                                             start=(k == 0), stop=(k == KC - 1))
                        for k in range(KC):
                            nc.tensor.matmul(pbias[:], encb_k[:, k:k + 1],
                                             wstage[:, k, :],
                                             start=(k == 0), stop=(k == KC - 1))
                        sl = slice(grp * 512, (grp + 1) * 512)
                        nc.vector.tensor_copy(W_combT[0:IN, sl], pb[:])
                        nc.vector.tensor_add(brow1[:, sl], pbias[:], bsum1[:, sl])
                    # bias row rides as contraction row 34 (DMA can hit
                    # the unaligned partition offset)
                    nc.sync.dma_start(W_combT[IN:IN + 1, :], brow1[:])

            # ============ fp8 weight builds (all resident) ============
            with tc.tile_pool(name="wpool", bufs=1) as wpool:
                w1 = wpool.tile([P, KPF, 2, G], FP8, tag="Whh1")
                w2h = wpool.tile([P, KPF, 2, G], FP8, tag="Whh2")
                w2b = wpool.tile([P, KC, G], BF16, tag="Wih2b")
                with nc.named_scope("build_W8"):
                    with tc.tile_pool(name="wrow1", bufs=6) as wrow, \
                         tc.tile_pool(name="wtr_ps1", bufs=3, space="PSUM") as wtr_ps:
                        _build_weight_T8(nc, W_hh1, w1, identr, wrow, wtr_ps)
                        _build_weight_T8(nc, W_hh2, w2h, identr, wrow, wtr_ps)
                        # W_ih2^T in bf16 (x512) for the batched A2 GEMM
                        for r in range(G // P):
                            wt = wrow.tile([P, H], F32R, tag="wrow")
                            nc.sync.dma_start(
                                wt[:], W_ih2[r * P:(r + 1) * P, :].bitcast(F32R))
                            for c in range(KC):
                                pt = wtr_ps.tile([P, P], F32R, tag="wtr")
                                nc.tensor.transpose(pt[:], wt[:, c * P:(c + 1) * P],
                                                    identr[:])
                                dst = w2b[:, c, r * P:(r + 1) * P]
                                if c % 2 == 0:
                                    nc.vector.tensor_scalar_mul(dst, pt[:],
                                                                WSCALE * HSCALE)
                                else:
                                    nc.scalar.activation(dst, pt[:], AF.Copy,
                                                         scale=WSCALE * HSCALE)

                # persistent state
                hT8 = state.tile([P, KPF, 2, NSLOT, P], FP8, tag="hT8_ring")
                hTb = state.tile([P, KC, NSLOT, BPC], HDT, tag="hTb_ring")
                c_st = state.tile([P, H], F32, tag="c_stack")
                nc.gpsimd.memset(hT8[:].bitcast(mybir.dt.uint8), 0.0)
                nc.gpsimd.memset(hTb[:].bitcast(mybir.dt.uint16), 0.0)
                nc.gpsimd.memset(c_st[:], 0.0)

                # decode operands
                decWT_f = misc.tile([P, KC], F32, tag="decWT_f")
                nc.sync.dma_start(decWT_f[:], dec_W.rearrange("o (c p) -> p (c o)", p=P))
                decWT = misc.tile([P, KC], HDT, tag="decWT")
                nc.vector.tensor_copy(decWT[:], decWT_f[:])
                decb_f = misc.tile([1, 1], F32, tag="decb_f")
                nc.sync.dma_start(decb_f[:], dec_b[None, :])
                decb_sb = misc.tile([1, 1], HDT, tag="decb")
                nc.vector.tensor_copy(decb_sb[:], decb_f[:])
                ones_f = misc.tile([1, BPC], F32, tag="ones_f")
                nc.gpsimd.memset(ones_f[:], 1.0)
                ones_bpc = misc.tile([1, BPC], HDT, tag="ones_bpc")
                nc.vector.tensor_copy(ones_bpc[:], ones_f[:])
                hT_last = misc.tile([P, KC, BPC], HDT, tag="hT_last")

                # ============ fused recurrence loop ============
                with nc.named_scope("loop"):
                    with tc.tile_pool(name="l_g", bufs=4) as gact, \
                         tc.tile_pool(name="l_a2", bufs=2) as a2pool, \
                         tc.tile_pool(name="l_h", bufs=2) as hpool, \
                         tc.tile_pool(name="l_pg", bufs=3, space="PSUM") as psum_g, \
                         tc.tile_pool(name="l_ptr", bufs=2, space="PSUM") as psum_tr:
                        pg_next = {}
                        a2_cur = None
                        for t in range(T + 2):
                            do_l1 = t < T
                            do_l2 = t >= 2
                            r0 = 0 if do_l1 else BPC
                            r1 = P if do_l2 else BPC
                            s_r = (t - 1) % NSLOT
                            s_w = t % NSLOT

                            # batched bf16 A2 block for L2-times (t-2, t-1):
                            # a2 = [h1_{t-2}; h1_{t-1}] @ (512*W_ih2)^T + 512*b2
                            if do_l2 and t % 2 == 0:
                                a2_cur = a2pool.tile([P, G], HDT, tag="a2sb")
                                s0 = (t - 2) % NSLOT
                                for chn in range(8):
                                    cs = slice(chn * 512, (chn + 1) * 512)
                                    pa = psum_tr.tile([P, 512], F32, tag="htr",
                                                      name="pa")
                                    for k in range(KC):
                                        nc.tensor.matmul(
                                            pa[:], hTb[:, k, s0:s0 + 2, :],
                                            w2b[:, k, cs],
                                            start=(k == 0), stop=(k == KC - 1),
                                            skip_group_check=True)
                                    nc.vector.tensor_add(a2_cur[:, cs], pa[:],
                                                         bias128_2[:, cs])
                            rh = BPC * (t % 2)  # a2 row-half for L2-time t-2

                            def mk_pgA(g_idx):
                                pgA = psum_g.tile([BPC, H], F32, tag="pg",
                                                  name=f"pgA{g_idx}")
                                for n2 in range(2):
                                    n = g_idx * 2 + n2
                                    nc.tensor.matmul(
                                        pgA[:, n2 * 512:(n2 + 1) * 512],
                                        xTa[:, t, :],
                                        W_combT[:, n * 512:(n + 1) * 512],
                                        start=True, stop=False,
                                        skip_group_check=True)
                                return pgA

                            def mm_l1(g_idx, pgA):
                                for n2 in range(2):
                                    n = g_idx * 2 + n2
                                    ch = slice(n2 * 512, (n2 + 1) * 512)
                                    wch = slice(n * 512, (n + 1) * 512)
                                    for kp in range(KPF):
                                        nc.tensor.matmul(
                                            pgA[:, ch],
                                            hT8[:, kp, :, s_r, 0:BPC],
                                            w1[:, kp, :, wch],
                                            start=False, stop=(kp == KPF - 1),
                                            perf_mode=DR,
                                            skip_group_check=True)

                            def mm_l2(g_idx):
                                pgB = psum_g.tile([BPC, H], F32, tag="pg",
                                                  name=f"pgB{g_idx}")
                                # a2 (+bias) preset via DVE, then accumulate
                                nc.vector.tensor_copy(
                                    pgB[:],
                                    a2_cur[rh:rh + BPC,
                                           g_idx * H:(g_idx + 1) * H])
                                for n2 in range(2):
                                    n = g_idx * 2 + n2
                                    ch = slice(n2 * 512, (n2 + 1) * 512)
                                    wch = slice(n * 512, (n + 1) * 512)
                                    for kp in range(KPF):
                                        nc.tensor.matmul(
                                            pgB[:, ch],
                                            hT8[:, kp, :, s_r, BPC:P],
                                            w2h[:, kp, :, wch],
                                            start=False, stop=(kp == KPF - 1),
                                            perf_mode=DR,
                                            skip_group_check=True)
                                return pgB

                            acts = {}

                            def do_gate(g_idx, func, name):
                                pgA = pg_next.pop(g_idx, None)
                                if do_l1:
                                    if pgA is None:
                                        pgA = mk_pgA(g_idx)
                                    mm_l1(g_idx, pgA)
                                pgB = mm_l2(g_idx) if do_l2 else None
                                at = gact.tile([P, H], HDT, tag="gact", name=name)
                                if do_l1:
                                    nc.scalar.activation(at[0:BPC], pgA[:], func,
                                                         bias=zb[0:BPC],
                                                         scale=INV_WSCALE)
                                if do_l2:
                                    nc.scalar.activation(at[BPC:P], pgB[:], func,
                                                         bias=zb[BPC:P],
                                                         scale=INV_WSCALE)
                                acts[g_idx] = at
                                return pgA, pgB

                            do_gate(0, AF.Sigmoid, "act_i")
                            do_gate(2, AF.Tanh, "act_g")
                            tmp = gact.tile([P, H], HDT, tag="gact", name="tmp")
                            nc.vector.tensor_mul(tmp[r0:r1], acts[0][r0:r1],
                                                 acts[2][r0:r1])

                            # gate f, then c update + tanh(c), in halves
                            pgA_f = pg_next.pop(1, None)
                            if do_l1:
                                if pgA_f is None:
                                    pgA_f = mk_pgA(1)
                                mm_l1(1, pgA_f)
                            pgB_f = mm_l2(1) if do_l2 else None
                            act_f = gact.tile([P, H], HDT, tag="gact", name="act_f")
                            tanh_c = gact.tile([P, H], HDT, tag="gact", name="tanh_c")
                            for hh in (1, 0):
                                sl = slice(hh * HF, (hh + 1) * HF)
                                if do_l1:
                                    nc.scalar.activation(act_f[0:BPC, sl],
                                                         pgA_f[:, sl], AF.Sigmoid,
                                                         bias=zb[0:BPC],
                                                         scale=INV_WSCALE)
                                if do_l2:
                                    nc.scalar.activation(act_f[BPC:P, sl],
                                                         pgB_f[:, sl], AF.Sigmoid,
                                                         bias=zb[BPC:P],
                                                         scale=INV_WSCALE)
                                nc.vector.tensor_mul(c_st[r0:r1, sl],
                                                     c_st[r0:r1, sl],
                                                     act_f[r0:r1, sl])
                                nc.vector.tensor_add(c_st[r0:r1, sl],
                                                     c_st[r0:r1, sl],
                                                     tmp[r0:r1, sl])
                                nc.scalar.activation(tanh_c[r0:r1, sl],
                                                     c_st[r0:r1, sl], AF.Tanh,
                                                     bias=zb[r0:r1])

                            # gate o + h, in halves
                            pgA_o = pg_next.pop(3, None)
                            if do_l1:
                                if pgA_o is None:
                                    pgA_o = mk_pgA(3)
                                mm_l1(3, pgA_o)
                            pgB_o = mm_l2(3) if do_l2 else None
                            act_o = gact.tile([P, H], HDT, tag="gact", name="act_o")
                            h_t = hpool.tile([P, H], HDT, tag="h_t")
                            for hh in (1, 0):
                                sl = slice(hh * HF, (hh + 1) * HF)
                                if do_l1:
                                    nc.scalar.activation(act_o[0:BPC, sl],
                                                         pgA_o[:, sl], AF.Sigmoid,
                                                         bias=zb[0:BPC],
                                                         scale=INV_WSCALE)
                                if do_l2:
                                    nc.scalar.activation(act_o[BPC:P, sl],
                                                         pgB_o[:, sl], AF.Sigmoid,
                                                         bias=zb[BPC:P],
                                                         scale=INV_WSCALE)
                                nc.vector.tensor_mul(h_t[r0:r1, sl],
                                                     act_o[r0:r1, sl],
                                                     tanh_c[r0:r1, sl])
                            # next-step L1 gate-i preload rides before the
                            # transposes: independent PE work in the tail
                            if t + 1 < T:
                                pgn = psum_g.tile([BPC, H], F32, tag="pg",
                                                  name="pgA0")
                                for n2 in range(2):
                                    nc.tensor.matmul(
                                        pgn[:, n2 * 512:(n2 + 1) * 512],
                                        xTa[:, t + 1, :],
                                        W_combT[:, n2 * 512:(n2 + 1) * 512],
                                        start=True, stop=False,
                                        skip_group_check=True)
                                pg_next[0] = pgn

                            # h^T transposes + ring writes (reversed: chunk
                            # 0, needed first next step, lands last)
                            if t <= T:
                                for k in range(KC - 1, -1, -1):
                                    pt = psum_tr.tile([P, P], HDT, tag="htr")
                                    nc.tensor.transpose(
                                        pt[:, r0:r1],
                                        h_t[r0:r1, k * P:(k + 1) * P],
                                        identh[r0:r1, r0:r1])
                                    nc.vector.tensor_scalar_mul(
                                        hT8[:, k // 2, k % 2, s_w, r0:r1],
                                        pt[:, r0:r1], HSCALE)
                                    if do_l1:
                                        # bf16 h1^T ring for the A2 GEMM
                                        nc.scalar.activation(
                                            hTb[:, k, s_w, :], pt[:, 0:BPC],
                                            AF.Copy)
                            else:
                                # final step: h2_{T-1}^T in bf16 for decode
                                for k in range(KC - 1, -1, -1):
                                    pt = psum_tr.tile([P, P], HDT, tag="htr")
                                    nc.tensor.transpose(
                                        pt[:, 0:BPC],
                                        h_t[BPC:P, k * P:(k + 1) * P],
                                        identh[BPC:P, BPC:P])
                                    nc.scalar.activation(
                                        hT_last[:, k, :], pt[:, 0:BPC], AF.Copy)

                        # decode: out = h2_{T-1} @ dec_W^T + dec_b
                        pd = psum_g.tile([1, BPC], F32, tag="pg", name="pdec")
                        for k in range(KC):
                            nc.tensor.matmul(pd[:], decWT[:, k:k + 1],
                                             hT_last[:, k, :],
                                             start=(k == 0), stop=False)
                        nc.tensor.matmul(pd[:], decb_sb[:], ones_bpc[:],
                                         start=False, stop=True)
                        osb = misc.tile([1, BPC], F32, tag="osb")
                        nc.vector.tensor_copy(osb[:], pd[:])
                        nc.sync.dma_start(out.rearrange("b o -> o b"), osb[:])

    nc.compile()
    return nc


_cached_nc = None
_cached_fn = None  # (jitted shard_map fn, in_names, out_names, out_shapes, zeros)


def _build_jitted(nc):
    """Same lowering as bass2jax.run_bass_via_pjrt, but the jitted
    executable is cached so repeat kernel() calls skip recompilation."""
    import jax
    from jax.sharding import Mesh, PartitionSpec
    from jax.experimental.shard_map import shard_map
    from concourse import bass2jax, mybir as _mybir

    bass2jax.install_neuronx_cc_hook()
    partition_name = nc.partition_id_tensor.name if nc.partition_id_tensor else None
    in_names, out_names, out_avals, zero_outs = [], [], [], []
    for alloc in nc.m.functions[0].allocations:
        if not isinstance(alloc, _mybir.MemoryLocationSet):
            continue
        name = alloc.memorylocations[0].name
        if alloc.kind == "ExternalInput":
            if name != partition_name:
                in_names.append(name)
        elif alloc.kind == "ExternalOutput":
            shape = tuple(alloc.tensor_shape)
            dtype = _mybir.dt.np(alloc.dtype)
            out_names.append(name)
            out_avals.append(jax.core.ShapedArray(shape, dtype))
            zero_outs.append(np.zeros(shape, dtype))
    n_params = len(in_names)
    n_outs = len(out_avals)
    all_in_names = list(in_names) + list(out_names)
    if partition_name is not None:
        all_in_names.append(partition_name)
    donate = tuple(range(n_params, n_params + n_outs))

    def _body(*args):
        operands = list(args)
        if partition_name is not None:
            operands.append(bass2jax.partition_id_tensor())
        outs = bass2jax._bass_exec_p.bind(
            *operands,
            out_avals=tuple(out_avals),
            in_names=tuple(all_in_names),
            out_names=tuple(out_names),
            lowering_input_output_aliases=(),
            sim_require_finite=True,
            sim_require_nnan=True,
            nc=nc,
        )
        return tuple(outs)

    devices = jax.devices()[:N_CORES]
    mesh = Mesh(np.asarray(devices), ("core",))
    in_specs = (PartitionSpec("core"),) * (n_params + n_outs)
    out_specs = (PartitionSpec("core"),) * n_outs
    fn = jax.jit(
        shard_map(_body, mesh=mesh, in_specs=in_specs, out_specs=out_specs,
                  check_rep=False),
        donate_argnums=donate, keep_unused=True,
    )
    out_shapes = [a.shape for a in out_avals]
    return fn, in_names, out_names, out_shapes, zero_outs


_dev_cache = {}  # name -> (digest, device_array)


def _to_device(name, arr):
    """Replicate-concat a weight to all cores and keep it on device across
    calls (keyed by content hash) so repeat kernel() calls only ship x."""
    import hashlib
    import jax
    d = hashlib.blake2b(arr.tobytes(), digest_size=16).digest()
    hit = _dev_cache.get(name)
    if hit is not None and hit[0] == d:
        return hit[1]
    conc = np.concatenate([arr] * N_CORES, axis=0)
    darr = jax.device_put(conc)
    _dev_cache[name] = (d, darr)
    return darr


def kernel(**inputs):
    global _cached_nc, _cached_fn
    if _cached_nc is None:
        _cached_nc = build(100)
        _cached_fn = _build_jitted(_cached_nc)
    fn, in_names, out_names, out_shapes, zero_outs = _cached_fn
    ins = {k: np.ascontiguousarray(np.asarray(v, dtype=np.float32))
           for k, v in inputs.items()}
    concat_in = []
    for name in in_names:
        if name == "x":
            concat_in.append(ins["x"])  # already [512, T, IN]; axis0 shards
        else:
            concat_in.append(_to_device(name, ins[name]))
    i = out_names.index("out")
    last_err = None
    for attempt in range(3):
        try:
            concat_zeros = [np.zeros((N_CORES * z.shape[0], *z.shape[1:]), z.dtype)
                            for z in zero_outs]
            out_arrs = fn(*concat_in, *concat_zeros)
            outp = np.asarray(out_arrs[i]).reshape(B, 1)
            return outp.astype(np.float32)
        except Exception as e:  # transient NRT_EXEC_UNIT_UNRECOVERABLE etc.
            last_err = e
            _dev_cache.clear()
            concat_in = []
            for name in in_names:
                if name == "x":
                    concat_in.append(ins["x"])
                else:
                    concat_in.append(_to_device(name, ins[name]))
    raise last_err
